# revision 1
# baseline (speedup 1.0000x reference)
"""CPC loss (nn_CPCLossV2) Trainium2 kernel.

Strategy (data-parallel over the n=4096 group axis, 512 groups/core x 8 cores):
  - Host: pure layout prep (transposes/slices of inputs, gather-index
    arithmetic + int16 wrapping). No reference math on host beyond the
    final mean of 8 per-core partial sums.
  - Device, per core:
      * cast the f32 embedding table to an internal fp16 copy (SWDGE
        cast-DMA, HBM->HBM)
      * predicts = hist_x @ W.T + b via PE (fp32, both orientations:
        [h, g] for scalar operands and [g, h] for the positive logit)
      * for each batch of 8 groups: transposed fp16 dma_gather pulls the
        256 negative rows/group as [h on partitions, j on free]
      * DVE tensor_scalar (4x fp16) multiplies by predicts[h] per group
      * PE "select-column" matmul (ones-column sliding window lhsT)
        simultaneously reduces over h (partitions) and routes group g's
        logits into psum row g%128 -> [128 groups, 256 negs] per block
      * softmax/logsumexp per block on DVE+ACT (exp with per-partition
        bias + accum_out), accumulate (lse - pos) per partition
      * final cross-partition sum via ones-matmul -> [1,1] partial
  - Host: loss = sum(partials) / 4096.
"""
import sys

if "/opt/trn_rl_repo" not in sys.path:
    sys.path.insert(0, "/opt/trn_rl_repo")

from contextlib import ExitStack

import numpy as np

import concourse.bass as bass
import concourse.bacc as bacc
import concourse.mybir as mybir
import concourse.tile as tile
from concourse.bass_utils import run_bass_kernel_spmd

# problem constants (hardcoded per harness contract)
N_GROUPS, K_POS, HID, M_NEG = 4096, 4, 256, 256
N_CORES = 8
GROUPS_PER_CALL = 8  # groups per dma_gather call (production setting)
TABLE_ROWS = N_GROUPS * K_POS          # 16384
CTX = (K_POS - 1) * HID                # 768
KC = CTX // 128                        # 6 contraction chunks
HC = HID // 128                        # 2 h chunks

F32 = mybir.dt.float32
F16 = mybir.dt.float16
I16 = mybir.dt.int16


def build_program(gpc: int, groups_per_call: int = 8, debug_stage: int = 5,
                  repeat: int = 1, sbuf_table: bool = False):
    """Build the per-core Tile program. gpc = groups per core.

    debug_stage (bisect aid): 1=predicts only, 2=+pos, 3=+gather/mul,
    4=+neg matmuls, 5=full softmax (production).
    repeat: run the negatives pipeline N times (timing instrumentation;
    results only valid for repeat=1).
    sbuf_table: keep the fp16 table resident in SBUF and gather from it
    (HBM sees the f32 table exactly once)."""
    assert gpc % 128 == 0 or gpc in (8, 16, 32, 64)
    nblocks = max(1, gpc // 128)
    block_sz = min(gpc, 128)
    ncalls = gpc // groups_per_call
    nidx = groups_per_call * M_NEG     # indices per gather call
    idx_cols_per_call = nidx // 16

    nc = bacc.Bacc("TRN2", target_bir_lowering=False, debug=False)

    emb = nc.dram_tensor("emb", [TABLE_ROWS, HID], F32, kind="ExternalInput")
    histxT = nc.dram_tensor("histxT", [CTX, gpc], F32, kind="ExternalInput")
    histy = nc.dram_tensor("histy", [gpc, HID], F32, kind="ExternalInput")
    wt = nc.dram_tensor("wt", [CTX, HID], F32, kind="ExternalInput")
    b_colT = nc.dram_tensor("b_colT", [128, HC], F32, kind="ExternalInput")
    b_bcast = nc.dram_tensor("b_bcast", [128, HID], F32, kind="ExternalInput")
    idx = nc.dram_tensor("idx", [128, ncalls * idx_cols_per_call], I16,
                         kind="ExternalInput")
    loss_out = nc.dram_tensor("loss", [1, 1], F32, kind="ExternalOutput")

    table16 = None if sbuf_table else nc.dram_tensor("table16", [TABLE_ROWS, HID], F16)

    with tile.TileContext(nc) as tc, ExitStack() as ctx:
        const_pool = ctx.enter_context(tc.tile_pool(name="const", bufs=1))
        gpool = ctx.enter_context(tc.tile_pool(name="gather", bufs=6))
        ppool = ctx.enter_context(tc.tile_pool(name="prod", bufs=12))
        spool = ctx.enter_context(tc.tile_pool(name="small", bufs=3))
        psum_neg = ctx.enter_context(tc.tile_pool(name="psn", bufs=3, space="PSUM"))
        psum_misc = ctx.enter_context(tc.tile_pool(name="psm", bufs=2, space="PSUM"))

        # --- fp16 table cast (SWDGE cast-DMA) ---
        if sbuf_table:
            # Partition-major layout: t16_sb[p, s, :] = emb[p*128 + s].
            # Each partition reads 128 consecutive rows (128 KB contiguous)
            # -> ~1 descriptor per partition instead of 1 per row. The host
            # compensates by permuting gather indices: token t = pi(v) =
            # (v % 128)*128 + v//128, so the gather (t%128 -> partition,
            # t//128 -> stripe) lands on emb[v].
            t16_sb = const_pool.tile([128, TABLE_ROWS // 128, HID], F16)
            nc.gpsimd.dma_start(
                t16_sb[:], emb.ap().rearrange("(p s) h -> p s h", p=128))
        else:
            nc.gpsimd.dma_start(table16.ap(), emb.ap())

        # --- constant / input loads ---
        idx_sb = const_pool.tile([128, ncalls * idx_cols_per_call], I16)
        nc.sync.dma_start(idx_sb[:], idx.ap())
        wt_sb = const_pool.tile([128, KC, HID], F32)
        nc.sync.dma_start(wt_sb[:], wt.ap().rearrange("(kc p) h -> p kc h", p=128))
        hx_sb = const_pool.tile([128, KC, gpc], F32)
        nc.sync.dma_start(hx_sb[:], histxT.ap().rearrange("(kc p) g -> p kc g", p=128))
        hy_sb = const_pool.tile([128, nblocks, HID], F32)
        nc.sync.dma_start(
            hy_sb[:block_sz, :, :],
            histy.ap().rearrange("(nb p) h -> p nb h", p=block_sz),
        )
        bcol_sb = const_pool.tile([128, HC], F32)
        nc.sync.dma_start(bcol_sb[:], b_colT.ap())
        bbc_sb = const_pool.tile([128, HID], F32)
        nc.sync.dma_start(bbc_sb[:], b_bcast.ap())

        # select matrix: zeros except col 127 = 1 (fp16)
        big = const_pool.tile([128, 256], F16)
        nc.vector.memset(big[:], 0.0)
        nc.vector.memset(big[:, 127:128], 1.0)
        ones_f32 = const_pool.tile([128, 1], F32)
        nc.vector.memset(ones_f32[:], 1.0)

        # --- predicts, orientation [h, g] (predT) ---
        predT = const_pool.tile([128, HC, gpc], F32)
        for hcx in range(HC):
            ps = psum_misc.tile([128, gpc], F32)
            for kcx in range(KC):
                nc.tensor.matmul(
                    ps[:],
                    wt_sb[:, kcx, hcx * 128:(hcx + 1) * 128],
                    hx_sb[:, kcx, :],
                    start=(kcx == 0), stop=(kcx == KC - 1),
                )
            nc.vector.tensor_scalar_add(predT[:, hcx, :], ps[:], bcol_sb[:, hcx:hcx + 1])

        # --- predicts, orientation [g, h] + positive logits ---
        pos_all = const_pool.tile([128, nblocks], F32)
        nc.vector.memset(pos_all[:], 0.0)
        for bx in range(nblocks if debug_stage >= 2 else 0):
            ps = psum_misc.tile([128, HID], F32)
            for kcx in range(KC):
                nc.tensor.matmul(
                    ps[:block_sz, :],
                    hx_sb[:, kcx, bx * block_sz:(bx + 1) * block_sz],
                    wt_sb[:, kcx, :],
                    start=(kcx == 0), stop=(kcx == KC - 1),
                )
            pred_b = spool.tile([128, HID], F32, tag="pred_b")
            nc.vector.tensor_add(pred_b[:block_sz, :], ps[:block_sz, :], bbc_sb[:block_sz, :])
            prodp = spool.tile([128, HID], F32, tag="prodp")
            nc.vector.tensor_mul(prodp[:block_sz, :], pred_b[:block_sz, :],
                                 hy_sb[:block_sz, bx, :])
            nc.vector.reduce_sum(pos_all[:block_sz, bx:bx + 1], prodp[:block_sz, :],
                                 axis=mybir.AxisListType.X)

        # --- negatives: gather -> scale -> select-reduce matmul ---
        acc = const_pool.tile([128, 1], F32)
        nc.vector.memset(acc[:], 0.0)
        exp_scratch = spool.tile([128, M_NEG], F32, tag="exps")

        psum_b = None
        for rep, call in [(rp, cl) for rp in range(repeat)
                          for cl in range(ncalls if debug_stage >= 3 else 0)]:
            gt = gpool.tile([128, HC, nidx], F16, tag="gt")
            idx_ap = idx_sb[:, call * idx_cols_per_call:(call + 1) * idx_cols_per_call]
            if sbuf_table:
                nc.gpsimd.dma_gather(
                    gt[:], t16_sb[:], idx_ap,
                    nidx, nidx, HID, transpose=True, single_packet=False,
                    sbuf_tokens_per_rank=128,
                    sbuf_free_dim_per_rank=HID * 2,
                )
            else:
                nc.gpsimd.dma_gather(
                    gt[:], table16.ap(), idx_ap,
                    nidx, nidx, HID, transpose=True, single_packet=False,
                )
            for g8 in range(groups_per_call):
                g = call * groups_per_call + g8
                bx, r = divmod(g, block_sz)
                if r == 0:
                    psum_b = psum_neg.tile([128, HC, M_NEG], F32, tag="psb")
                prod = ppool.tile([128, HC, M_NEG], F16, tag="prod")
                for hcx in range(HC):
                    nc.vector.tensor_scalar_mul(
                        prod[:, hcx, :],
                        gt[:, hcx, g8 * M_NEG:(g8 + 1) * M_NEG],
                        predT[:, hcx, g:g + 1],
                    )
                if debug_stage < 4:
                    continue
                nc.tensor.matmul(
                    psum_b[:block_sz, :, :],
                    big[:, 127 - r:127 - r + block_sz],
                    prod[:, :, :],
                    start=(r == 0), stop=(r == block_sz - 1),
                )
                if r == block_sz - 1 and debug_stage >= 5 and rep == 0:
                    # --- combine the two h-chunk partial sums -> full logits ---
                    negs_sb = spool.tile([128, M_NEG], F32, tag="negs")
                    nc.scalar.copy(negs_sb[:block_sz, :], psum_b[:block_sz, 0, :])
                    nc.vector.tensor_add(negs_sb[:block_sz, :], negs_sb[:block_sz, :],
                                         psum_b[:block_sz, 1, :])
                    # --- block softmax / logsumexp ---
                    mx = spool.tile([128, 1], F32, tag="mx")
                    nc.vector.reduce_max(mx[:block_sz, :], negs_sb[:block_sz, :],
                                         axis=mybir.AxisListType.X)
                    mx2 = spool.tile([128, 1], F32, tag="mx2")
                    nc.vector.tensor_max(mx2[:block_sz, :], mx[:block_sz, :],
                                         pos_all[:block_sz, bx:bx + 1])
                    nmx2 = spool.tile([128, 1], F32, tag="nmx2")
                    nc.vector.tensor_scalar_mul(nmx2[:block_sz, :], mx2[:block_sz, :], -1.0)
                    sumexp = spool.tile([128, 1], F32, tag="sumexp")
                    nc.scalar.activation(
                        exp_scratch[:block_sz, :], negs_sb[:block_sz, :],
                        mybir.ActivationFunctionType.Exp,
                        bias=nmx2[:block_sz, :], scale=1.0,
                        accum_out=sumexp[:block_sz, :],
                    )
                    expp = spool.tile([128, 1], F32, tag="expp")
                    nc.scalar.activation(
                        expp[:block_sz, :], pos_all[:block_sz, bx:bx + 1],
                        mybir.ActivationFunctionType.Exp,
                        bias=nmx2[:block_sz, :], scale=1.0,
                    )
                    denom = spool.tile([128, 1], F32, tag="denom")
                    nc.vector.tensor_add(denom[:block_sz, :], sumexp[:block_sz, :],
                                         expp[:block_sz, :])
                    logd = spool.tile([128, 1], F32, tag="logd")
                    nc.scalar.activation(logd[:block_sz, :], denom[:block_sz, :],
                                         mybir.ActivationFunctionType.Ln)
                    lse = spool.tile([128, 1], F32, tag="lse")
                    nc.vector.tensor_add(lse[:block_sz, :], logd[:block_sz, :],
                                         mx2[:block_sz, :])
                    li = spool.tile([128, 1], F32, tag="li")
                    nc.vector.tensor_sub(li[:block_sz, :], lse[:block_sz, :],
                                         pos_all[:block_sz, bx:bx + 1])
                    nc.vector.tensor_add(acc[:block_sz, :], acc[:block_sz, :],
                                         li[:block_sz, :])

        # --- cross-partition sum -> [1,1] ---
        ps_fin = psum_misc.tile([1, 1], F32)
        nc.tensor.matmul(ps_fin[:], ones_f32[:block_sz, :],
                         acc[:block_sz, :], start=True, stop=True)
        out_sb = spool.tile([1, 1], F32, tag="out")
        nc.vector.tensor_copy(out_sb[:], ps_fin[:])
        nc.sync.dma_start(loss_out.ap(), out_sb[:])

    nc.compile()
    return nc


def prep_core_inputs(embeddings, W, b, neg_perm, core, gpc, groups_per_call=8,
                     permute_idx=True):
    """Host-side layout prep for one core's in_map.

    permute_idx must match the program's sbuf_table setting (True for the
    production partition-major SBUF table layout)."""
    n = N_GROUPS
    k = K_POS
    g0 = core * gpc
    e = embeddings.reshape(n, k, HID)
    hist_x = e[g0:g0 + gpc, :k - 1, :].reshape(gpc, CTX)
    histxT = np.ascontiguousarray(hist_x.T)
    histy = np.ascontiguousarray(e[g0:g0 + gpc, k - 1, :])
    wt = np.ascontiguousarray(W.T)
    b_colT = np.ascontiguousarray(b.reshape(HC, 128).T)
    b_bcast = np.ascontiguousarray(np.broadcast_to(b, (128, HID)))

    gi = np.arange(g0, g0 + gpc, dtype=np.int64)[:, None]
    np_perm = neg_perm[g0:g0 + gpc].astype(np.int64)
    neg_idx = np_perm + np.where(np_perm >= gi * k, k, 0)
    assert neg_idx.max() < TABLE_ROWS
    if permute_idx:
        # token id for the partition-major SBUF table (see build_program)
        neg_idx = (neg_idx % 128) * 128 + neg_idx // 128
    neg_idx = neg_idx.astype(np.int16)

    ncalls = gpc // groups_per_call
    nidx = groups_per_call * M_NEG
    # per call: seq [nidx] -> wrapped [16, nidx/16] (g -> (g%16, g//16)),
    # replicated 8x across 128 partitions
    seq = neg_idx.reshape(ncalls, nidx)
    wrapped = seq.reshape(ncalls, nidx // 16, 16).transpose(0, 2, 1)  # [ncalls, 16, nidx/16]
    rep = np.tile(wrapped, (1, 8, 1))                                 # [ncalls, 128, nidx/16]
    idx_all = np.ascontiguousarray(rep.transpose(1, 0, 2).reshape(128, ncalls * (nidx // 16)))

    return {
        "emb": np.ascontiguousarray(embeddings, dtype=np.float32),
        "histxT": histxT.astype(np.float32),
        "histy": histy.astype(np.float32),
        "wt": wt.astype(np.float32),
        "b_colT": b_colT.astype(np.float32),
        "b_bcast": b_bcast.astype(np.float32),
        "idx": idx_all,
    }


_PROGRAM_CACHE = {}


def _get_program(gpc):
    if gpc not in _PROGRAM_CACHE:
        _PROGRAM_CACHE[gpc] = build_program(
            gpc, groups_per_call=GROUPS_PER_CALL, sbuf_table=True)
    return _PROGRAM_CACHE[gpc]


def kernel(embeddings, W, b, target, neg_perm, k_pos_samples):
    embeddings = np.asarray(embeddings, dtype=np.float32)
    W = np.asarray(W, dtype=np.float32)
    b = np.asarray(b, dtype=np.float32)
    neg_perm = np.asarray(neg_perm)
    assert int(k_pos_samples) == K_POS
    assert embeddings.shape == (TABLE_ROWS, HID)

    gpc = N_GROUPS // N_CORES
    nc = _get_program(gpc)
    in_maps = [
        prep_core_inputs(embeddings, W, b, neg_perm, core, gpc, GROUPS_PER_CALL)
        for core in range(N_CORES)
    ]
    res = run_bass_kernel_spmd(nc, in_maps, list(range(N_CORES)))
    total = sum(float(r["loss"][0, 0]) for r in res.results)
    return np.float32(total / N_GROUPS)



# revision 5
# speedup vs baseline: 2.5332x; 2.5332x over previous
"""CPC loss (nn_CPCLossV2) Trainium2 kernel — dense-logits formulation.

Strategy (data-parallel over the n=4096 group axis, 512 groups/core x 8 cores):
  - Host: pure layout prep (transposes/slices, gather-index arithmetic, and a
    log-count mask encoding of neg_perm as fp8). No reference math on host
    beyond the final mean of 8 per-core partial sums.
  - Device, per core, instead of gathering 512x256 negative embedding rows
    (67 MB of SWDGE traffic -> old bottleneck), compute ALL logits:
      * cast the f32 embedding table (host-transposed to [h, v]) to an fp8
        SBUF copy via SWDGE cast-DMA; same for the fp8 log-count mask
      * predicts via PE (fp16), quantized to fp8 in [h, g] orientation
      * L[g, v] = predicts . emb_v for all 16384 candidates via fp8
        DoubleRow matmuls (K=256 contracted in one instruction), plus an
        identity-weight DoubleRow matmul that adds lnc[g, v] = ln(count) for
        selected candidates and -240 for non-selected -> masked logits in PSUM
      * ACT evicts each 4-bank PSUM superchunk with Exp(x - 30) + accum_out,
        yielding sum_j count[g,j] * exp(l_j - 30) without any gather
      * lse = ln(denom + exp(pos - 30)) + 30; loss_i = lse - pos
  - Host: loss = sum(partials) / 4096.
"""
import sys

if "/opt/trn_rl_repo" not in sys.path:
    sys.path.insert(0, "/opt/trn_rl_repo")

from contextlib import ExitStack

import numpy as np
import ml_dtypes

import concourse.bass as bass
import concourse.bacc as bacc
import concourse.mybir as mybir
import concourse.tile as tile
from concourse.ap import AP
from concourse.bass_utils import run_bass_kernel_spmd

# problem constants (hardcoded per harness contract)
N_GROUPS, K_POS, HID, M_NEG = 4096, 4, 256, 256
N_CORES = 8
TABLE_ROWS = N_GROUPS * K_POS          # 16384 candidate rows (V)
V = TABLE_ROWS
CTX = (K_POS - 1) * HID                # 768
KC = CTX // 128                        # 6 contraction chunks for predicts
HC = HID // 128                        # 2 h chunks
CHUNK = 512                            # psum bank width (f32)
SUPER = 2048                           # ACT eviction granularity (4 banks)
NSUPER = V // SUPER                    # 8
MASKW = V + CHUNK                      # mask width incl zero pad columns
MASK_NEG = -240.0                      # fp8e4m3 representable "minus inf"
BIAS = -30.0                           # exp stability shift

F32 = mybir.dt.float32
F16 = mybir.dt.float16
F8 = mybir.dt.float8e4
NP_F8 = ml_dtypes.float8_e4m3
DR = mybir.MatmulPerfMode.DoubleRow


def _mask_rhs(lnc_sb, b, col0):
    """AP view [128, 2, CHUNK] of the mask tile: k-tile 0 -> lnc[:, b,
    col0:col0+CHUNK], k-tile 1 -> the zero pad columns [V, V+CHUNK).
    Feeds the identity DoubleRow matmul (second k-tile weights are zero
    anyway, but point it at real zeros so any dtype garbage is avoided)."""
    base = lnc_sb[:, b, 0:CHUNK]
    p_dim = list(base.ap[0])
    return AP(base.tensor, base.offset + col0, [p_dim, [V - col0, 2], [1, CHUNK]])


def build_program(gpc: int):
    """Build the per-core Tile program. gpc = groups per core (512)."""
    nblocks = gpc // 128

    nc = bacc.Bacc("TRN2", target_bir_lowering=False, debug=False)

    embT = nc.dram_tensor("embT", [HID, V], F32, kind="ExternalInput")
    lnc8 = nc.dram_tensor("lnc8", [gpc, V], F8, kind="ExternalInput")
    i2 = nc.dram_tensor("i2", [128, 2 * 128], F8, kind="ExternalInput")
    histxT = nc.dram_tensor("histxT", [CTX, gpc], F32, kind="ExternalInput")
    wt = nc.dram_tensor("wt", [CTX, HID], F32, kind="ExternalInput")
    histy = nc.dram_tensor("histy", [gpc, HID], F32, kind="ExternalInput")
    b_colT = nc.dram_tensor("b_colT", [128, HC], F32, kind="ExternalInput")
    b_bcast = nc.dram_tensor("b_bcast", [128, HID], F32, kind="ExternalInput")
    loss_out = nc.dram_tensor("loss", [1, 1], F32, kind="ExternalOutput")

    with tile.TileContext(nc) as tc, ExitStack() as ctx:
        cpool = ctx.enter_context(tc.tile_pool(name="const", bufs=1))
        spool = ctx.enter_context(tc.tile_pool(name="small", bufs=3))
        epool = ctx.enter_context(tc.tile_pool(name="expsc", bufs=2))
        psum = ctx.enter_context(tc.tile_pool(name="ps", bufs=2, space="PSUM"))

        # --- small loads (HWDGE queue) ---
        i2_sb = cpool.tile([128, 2, 128], F8)
        nc.sync.dma_start(i2_sb[:], i2.ap().rearrange("p (two c) -> p two c", two=2))
        bcol = cpool.tile([128, HC], F32)
        nc.sync.dma_start(bcol[:], b_colT.ap())
        bbc = cpool.tile([128, HID], F32)
        nc.sync.dma_start(bbc[:], b_bcast.ap())

        # --- fp16 cast loads for the predicts matmuls (SWDGE cast path) ---
        wt16 = cpool.tile([128, KC, HID], F16)
        nc.gpsimd.dma_start(wt16[:], wt.ap().rearrange("(kc p) h -> p kc h", p=128))
        hx16 = cpool.tile([128, KC, gpc], F16)
        nc.gpsimd.dma_start(hx16[:], histxT.ap().rearrange("(kc p) g -> p kc g", p=128))

        # --- mask tile: zero pad cols + streamed chunks (HWDGE queue) ---
        lnc_sb = cpool.tile([128, nblocks, MASKW], F8)
        nc.vector.memset(lnc_sb[:, :, V:], 0.0)
        lnc_re = lnc8.ap().rearrange("(nb p) v -> p nb v", p=128)
        nc.sync.dma_start(lnc_sb[:, :, 0:SUPER], lnc_re[:, :, 0:SUPER])

        # --- fp8 embedding table (transposed layout), streamed chunks ---
        embT8 = cpool.tile([128, HC, V], F8)
        embT_re = embT.ap().rearrange("(hc p) v -> p hc v", p=128)
        for s in range(NSUPER):
            sl = slice(s * SUPER, (s + 1) * SUPER)
            nc.gpsimd.dma_start(embT8[:, :, sl], embT_re[:, :, sl])
            if s >= 1:
                nc.sync.dma_start(lnc_sb[:, :, sl], lnc_re[:, :, sl])
        hy = cpool.tile([128, nblocks, HID], F32)
        nc.sync.dma_start(hy[:], histy.ap().rearrange("(nb p) h -> p nb h", p=128))

        # --- predicts, orientation [h, g], quantized fp8 (lhsT for L) ---
        predT8 = cpool.tile([128, HC, gpc], F8)
        for hcx in range(HC):
            ps = psum.tile([128, 4, CHUNK], F32, tag="ps")
            for kcx in range(KC):
                nc.tensor.matmul(
                    ps[:, 0, :],
                    wt16[:, kcx, hcx * 128:(hcx + 1) * 128],
                    hx16[:, kcx, :],
                    start=(kcx == 0), stop=(kcx == KC - 1),
                )
            nc.vector.tensor_scalar_add(predT8[:, hcx, :], ps[:, 0, :],
                                        bcol[:, hcx:hcx + 1])

        # --- predicts, orientation [g, h] (f32) + positive logits ---
        pos_all = cpool.tile([128, nblocks], F32)
        for bx in range(nblocks):
            ps = psum.tile([128, 4, CHUNK], F32, tag="ps")
            for kcx in range(KC):
                nc.tensor.matmul(
                    ps[:, 0, 0:HID],
                    hx16[:, kcx, bx * 128:(bx + 1) * 128],
                    wt16[:, kcx, :],
                    start=(kcx == 0), stop=(kcx == KC - 1),
                )
            pred_b = spool.tile([128, HID], F32, tag="pred_b")
            nc.vector.tensor_add(pred_b[:], ps[:, 0, 0:HID], bbc[:])
            prodp = spool.tile([128, HID], F32, tag="prodp")
            nc.vector.tensor_mul(prodp[:], pred_b[:], hy[:, bx, :])
            nc.vector.reduce_sum(pos_all[:, bx:bx + 1], prodp[:],
                                 axis=mybir.AxisListType.X)
        bias_t = cpool.tile([128, 1], F32)
        nc.vector.memset(bias_t[:], BIAS)
        pose = cpool.tile([128, nblocks], F32)
        nc.scalar.activation(pose[:], pos_all[:],
                             mybir.ActivationFunctionType.Exp, bias=bias_t[:])

        # --- dense masked logits + exp-accumulate ---
        sums3 = cpool.tile([128, nblocks, NSUPER], F32)
        for s in range(NSUPER):
            for b in range(nblocks):
                ps = psum.tile([128, 4, CHUNK], F32, tag="ps")
                for c4 in range(4):
                    col0 = s * SUPER + c4 * CHUNK
                    nc.tensor.matmul(
                        ps[:, c4, :],
                        predT8[:, :, b * 128:(b + 1) * 128],
                        embT8[:, :, col0:col0 + CHUNK],
                        start=True, stop=False, perf_mode=DR,
                    )
                    nc.tensor.matmul(
                        ps[:, c4, :],
                        i2_sb[:],
                        _mask_rhs(lnc_sb, b, col0),
                        start=False, stop=True, perf_mode=DR,
                    )
                es = epool.tile([128, 4, CHUNK], F32, tag="es")
                nc.scalar.activation(
                    es[:], ps[:],
                    mybir.ActivationFunctionType.Exp,
                    bias=bias_t[:], scale=1.0,
                    accum_out=sums3[:, b, s:s + 1],
                )

        # --- logsumexp tail ---
        denom = spool.tile([128, nblocks, 1], F32, tag="denom")
        nc.vector.reduce_sum(denom[:], sums3[:], axis=mybir.AxisListType.X)
        denom2 = spool.tile([128, nblocks], F32, tag="denom2")
        nc.vector.tensor_add(denom2[:], denom[:, :, 0], pose[:])
        logd = spool.tile([128, nblocks], F32, tag="logd")
        nc.scalar.activation(logd[:], denom2[:], mybir.ActivationFunctionType.Ln)
        li = spool.tile([128, nblocks], F32, tag="li")
        nc.vector.tensor_sub(li[:], logd[:], pos_all[:])
        li2 = spool.tile([128, nblocks], F32, tag="li2")
        nc.vector.tensor_scalar_add(li2[:], li[:], -BIAS)
        acc = spool.tile([128, 1], F32, tag="acc")
        nc.vector.reduce_sum(acc[:], li2[:], axis=mybir.AxisListType.X)

        ones_f32 = cpool.tile([128, 1], F32)
        nc.vector.memset(ones_f32[:], 1.0)
        ps_fin = psum.tile([128, 4, CHUNK], F32, tag="ps")
        nc.tensor.matmul(ps_fin[0:1, 0, 0:1], ones_f32[:], acc[:],
                         start=True, stop=True)
        out_sb = spool.tile([1, 1], F32, tag="out")
        nc.vector.tensor_copy(out_sb[:], ps_fin[0:1, 0, 0:1])
        nc.sync.dma_start(loss_out.ap(), out_sb[:])

    nc.compile()
    return nc


_LNC_LUT = None


def _lnc_lut(maxc):
    global _LNC_LUT
    if _LNC_LUT is None or len(_LNC_LUT) <= maxc:
        lut = np.full(max(maxc + 1, 16), MASK_NEG, np.float32)
        lut[1:] = np.log(np.arange(1, len(lut)))
        _LNC_LUT = lut.astype(NP_F8)
    return _LNC_LUT


def prep_core_inputs(embeddings, W, b, neg_perm, core, gpc, embT=None):
    """Host-side layout prep for one core's in_map."""
    n, k = N_GROUPS, K_POS
    g0 = core * gpc
    e = embeddings.reshape(n, k, HID)
    hist_x = e[g0:g0 + gpc, :k - 1, :].reshape(gpc, CTX)
    histxT = np.ascontiguousarray(hist_x.T)
    histy = np.ascontiguousarray(e[g0:g0 + gpc, k - 1, :])
    wt = np.ascontiguousarray(W.T)
    b_colT = np.ascontiguousarray(b.reshape(HC, 128).T)
    b_bcast = np.ascontiguousarray(np.broadcast_to(b, (128, HID)))
    if embT is None:
        embT = np.ascontiguousarray(embeddings.T)

    # sorted negatives for group i are {0..i*k-1} u {(i+1)*k..n*k-1};
    # candidate position j maps to global index j (j < i*k) else j + k
    gi = np.arange(g0, g0 + gpc, dtype=np.int64)[:, None]
    npp = neg_perm[g0:g0 + gpc].astype(np.int64)
    neg_idx = npp + np.where(npp >= gi * k, k, 0)
    assert neg_idx.max() < V
    flat = (np.arange(gpc, dtype=np.int64)[:, None] * V + neg_idx).ravel()
    cnt = np.bincount(flat, minlength=gpc * V).reshape(gpc, V)
    lnc = _lnc_lut(int(cnt.max()))[cnt]

    i2 = np.zeros((128, 2, 128), np.float32)
    i2[:, 0, :] = np.eye(128, dtype=np.float32)

    return {
        "embT": embT,
        "lnc8": lnc,
        "i2": i2.reshape(128, 256).astype(NP_F8),
        "histxT": histxT.astype(np.float32),
        "wt": wt.astype(np.float32),
        "histy": histy.astype(np.float32),
        "b_colT": b_colT.astype(np.float32),
        "b_bcast": b_bcast.astype(np.float32),
    }


_PROGRAM_CACHE = {}


def _get_program(gpc):
    if gpc not in _PROGRAM_CACHE:
        _PROGRAM_CACHE[gpc] = build_program(gpc)
    return _PROGRAM_CACHE[gpc]


def kernel(embeddings, W, b, target, neg_perm, k_pos_samples):
    embeddings = np.asarray(embeddings, dtype=np.float32)
    W = np.asarray(W, dtype=np.float32)
    b = np.asarray(b, dtype=np.float32)
    neg_perm = np.asarray(neg_perm)
    assert int(k_pos_samples) == K_POS
    assert embeddings.shape == (TABLE_ROWS, HID)

    gpc = N_GROUPS // N_CORES
    nc = _get_program(gpc)
    embT = np.ascontiguousarray(embeddings.T)
    in_maps = [
        prep_core_inputs(embeddings, W, b, neg_perm, core, gpc, embT=embT)
        for core in range(N_CORES)
    ]
    res = run_bass_kernel_spmd(nc, in_maps, list(range(N_CORES)))
    total = sum(float(r["loss"][0, 0]) for r in res.results)
    return np.float32(total / N_GROUPS)
